# revision 21
# baseline (speedup 1.0000x reference)
"""Trainium2 Bass kernel for gnn_message_passing (nn_CMP_32427003085021).

reference semantics:
    pooled_pos[t] = sum_{e: tgt=t, typ>0} feats[src_e]
    pooled_neg[t] = sum_{e: tgt=t, typ<0} feats[src_e]
    combined = [feats; pooled_pos; pooled_neg]            # [N, 48, 32, 32]
    h = lrelu(snconv(combined, W1/s1) + b1, 0.2)
    h = lrelu(snconv(h, W2/s2) + b2, 0.2)
    out = feats + h

Distribution: 8 cores, 250 target-nodes each (data parallel over nodes).
Full feats replicated per core (local HBM) so every gather is core-local.
Pooling is done entirely by SWDGE indirect accumulate-gather DMAs
(compute_op=add, OOB-padded index rounds) writing into padded SBUF images.
Convs are kn2row: 9 shifted matmuls (fp32r, full PE column rate) per output
half, with 4 nodes running concurrently on disjoint 32-row/32-col PE
sub-array tiles (diagonal tile_position via base partitions).
"""
import sys

if "/opt/trn_rl_repo" not in sys.path:
    sys.path.insert(0, "/opt/trn_rl_repo")

import numpy as np

from concourse import bass, bacc, mybir
import concourse.tile as tile
from concourse.bass_utils import run_bass_kernel_spmd

# problem constants (hardcoded per contract)
N_NODES = 2000
N_EDGES = 8000
C = 16
H = W = 32
PIX = H * W          # 1024
HP = H + 2           # 34
N_CORES = 8
NPC = N_NODES // N_CORES        # 250
NPC_PAD = 252                   # multiple of 4
NG = NPC_PAD // 4               # 63 groups of 4 nodes
ROWS = N_NODES * C              # gather row space: feats as [32000, 1024]
ZROW = ROWS                     # appended all-zeros row; pad gathers add 0

F32 = mybir.dt.float32
F32R = mybir.dt.float32r
BF16 = mybir.dt.bfloat16
I32 = mybir.dt.int32

_compiled = None  # (nc, R) cache


def _l2n(x):
    return x / (np.linalg.norm(x) + 1e-12)


def _sigma(Wc, u):
    """One power iteration, mirrors reference fp32 math."""
    O = Wc.shape[0]
    Wm = Wc.reshape(O, -1).astype(np.float32)
    v = _l2n(Wm.T @ u.astype(np.float32))
    u2 = _l2n(Wm @ v)
    return float(u2 @ (Wm @ v))


def _build(R):
    """Build the SPMD program. R = max gather rounds per (node, sign)."""
    nc = bacc.Bacc("TRN2", target_bir_lowering=False, debug=False)
    feats_rows = nc.dram_tensor("feats_rows", [ROWS + 1, PIX], BF16, kind="ExternalInput")
    feats_loc = nc.dram_tensor("feats_loc", [NPC_PAD, C, HP * HP], BF16, kind="ExternalInput")
    feats_res = nc.dram_tensor("feats_res", [NPC_PAD, C, PIX], F32, kind="ExternalInput")
    gidx = nc.dram_tensor("gidx", [128, NG * R], I32, kind="ExternalInput")
    w1p = nc.dram_tensor("w1p", [128, 9 * 32], BF16, kind="ExternalInput")
    w1f = nc.dram_tensor("w1f", [128, 9 * 32], BF16, kind="ExternalInput")
    w2w = nc.dram_tensor("w2w", [128, 9 * 32], BF16, kind="ExternalInput")
    w1b = nc.dram_tensor("w1b", [128, 1], F32, kind="ExternalInput")
    w2b = nc.dram_tensor("w2b", [128, 1], F32, kind="ExternalInput")
    out = nc.dram_tensor("out", [NPC_PAD, C, PIX], F32, kind="ExternalOutput")

    BUFS = 4
    with tile.TileContext(nc) as tc:
        with (
            tc.tile_pool(name="const", bufs=1) as cpool,
            tc.tile_pool(name="work", bufs=1) as wpool,
            tc.tile_pool(name="psum", bufs=2, space="PSUM") as pspool,
        ):
            # ---- constants resident in SBUF ----
            idx_t = cpool.tile([128, NG * R], I32)
            nc.sync.dma_start(out=idx_t[:], in_=gidx[:])
            w1p_t = cpool.tile([128, 9, 32], BF16)
            nc.sync.dma_start(out=w1p_t[:], in_=w1p[:])
            w1f_t = cpool.tile([128, 9, 32], BF16)
            nc.sync.dma_start(out=w1f_t[:], in_=w1f[:])
            w2_t = cpool.tile([128, 9, 32], BF16)
            nc.sync.dma_start(out=w2_t[:], in_=w2w[:])
            w1b_t = cpool.tile([128, 1], F32)
            nc.sync.dma_start(out=w1b_t[:], in_=w1b[:])
            w2b_t = cpool.tile([128, 1], F32)
            nc.sync.dma_start(out=w2b_t[:], in_=w2b[:])
            alpha_t = cpool.tile([128, 1], F32)
            nc.vector.memset(alpha_t[:], 0.2)

            offsets = [(dy, dx) for dy in (-1, 0, 1) for dx in (-1, 0, 1)]

            for g in range(NG):
                comb = wpool.tile([128, HP, HP], BF16, tag="comb", name="comb", bufs=BUFS)
                comb_raw = wpool.tile([128, PIX], BF16, tag="comb_raw", name="comb_raw", bufs=BUFS)
                ft = wpool.tile([128, HP, HP], BF16, tag="ft", name="ft", bufs=BUFS)
                fres = wpool.tile([128, PIX], F32, tag="fres", name="fres", bufs=BUFS)
                h1 = wpool.tile([128, HP, HP], BF16, tag="h1", name="h1", bufs=BUFS)
                h2t = wpool.tile([128, PIX], F32, tag="h2", name="h2", bufs=3)
                outt = wpool.tile([128, PIX], F32, tag="outt", name="outt", bufs=3)
                # zero padded tiles (comb_raw accumulates from zero)
                nc.vector.memset(comb[:], 0.0)
                nc.vector.memset(comb_raw[:], 0.0)
                nc.vector.memset(h1[:], 0.0)
                nc.vector.memset(fres[:], 0.0)

                # self features for the 4 nodes (source pre-padded on host)
                for c in range(4):
                    nc.scalar.dma_start(
                        out=ft[32 * c : 32 * c + C, :, :],
                        in_=feats_loc[4 * g + c],
                    )
                    nc.scalar.dma_start(
                        out=fres[32 * c : 32 * c + C, :],
                        in_=feats_res[4 * g + c],
                    )

                # pooling: R rounds of indirect accumulate-gather
                for k in range(R):
                    col = g * R + k
                    nc.gpsimd.indirect_dma_start(
                        out=comb_raw[:],
                        out_offset=None,
                        in_=feats_rows[:],
                        in_offset=bass.IndirectOffsetOnAxis(
                            ap=idx_t[:, col : col + 1], axis=0
                        ),
                        compute_op=mybir.AluOpType.add,
                    )

                # pooled -> padded interior (indirect DMA needs contiguous dest)
                nc.vector.tensor_copy(
                    out=comb[:, 1 : 1 + H, 1 : 1 + W], in_=comb_raw[:]
                )

                # ---- conv1 into ps1 (K=32 pooled + K=16 feats + bias) ----
                ps1 = pspool.tile([128, PIX], F32, tag="ps1", name="ps1")
                for h in range(2):
                    for i, (dy, dx) in enumerate(offsets):
                        r0 = 16 * h + dy + 1
                        c0 = dx + 1
                        for c in range(4):
                            b = 32 * c
                            nc.tensor.matmul(
                                out=ps1[b : b + 32, 512 * h : 512 * h + 512],
                                lhsT=w1p_t[b : b + 32, i, :],
                                rhs=comb[b : b + 32, r0 : r0 + 16, c0 : c0 + W],
                                start=(i == 0),
                                stop=False,
                                tile_position=(b, b),
                                skip_group_check=True,
                            )
                    for i, (dy, dx) in enumerate(offsets):
                        r0 = 16 * h + dy + 1
                        c0 = dx + 1
                        for c in range(4):
                            b = 32 * c
                            nc.tensor.matmul(
                                out=ps1[b : b + 32, 512 * h : 512 * h + 512],
                                lhsT=w1f_t[b : b + C, i, :],
                                rhs=ft[b : b + C, r0 : r0 + 16, c0 : c0 + W],
                                start=False,
                                stop=(i == 8),
                                tile_position=(b, b),
                                skip_group_check=True,
                            )
                    # evac half h with leaky relu into padded h1 interior
                    nc.scalar.activation(
                        out=h1[:, 1 + 16 * h : 17 + 16 * h, 1 : 1 + W],
                        in_=ps1[:, 512 * h : 512 * h + 512],
                        func=mybir.ActivationFunctionType.Prelu,
                        bias=w1b_t[:],
                        scale=1.0,
                        alpha=alpha_t[:],
                    )

                # ---- conv2 into ps2 (K=32 + bias) ----
                ps2 = pspool.tile([128, PIX], F32, tag="ps2", name="ps2")
                for h in range(2):
                    for i, (dy, dx) in enumerate(offsets):
                        r0 = 16 * h + dy + 1
                        c0 = dx + 1
                        for c in range(4):
                            b = 32 * c
                            nc.tensor.matmul(
                                out=ps2[b : b + 32, 512 * h : 512 * h + 512],
                                lhsT=w2_t[b : b + 32, i, :],
                                rhs=h1[b : b + 32, r0 : r0 + 16, c0 : c0 + W],
                                start=(i == 0),
                                stop=(i == 8),
                                tile_position=(b, b),
                                skip_group_check=True,
                            )
                # leaky on full partitions (junk rows unused later)
                nc.scalar.activation(
                    out=h2t[:],
                    in_=ps2[:],
                    func=mybir.ActivationFunctionType.Prelu,
                    bias=w2b_t[:],
                    scale=1.0,
                    alpha=alpha_t[:],
                )
                # residual: out = feats + h  (ft unused rows are zero)
                nc.vector.tensor_tensor(
                    out=outt[:],
                    in0=h2t[:],
                    in1=fres[:],
                    op=mybir.AluOpType.add,
                )
                for c in range(4):
                    nc.scalar.dma_start(
                        out=out[4 * g + c],
                        in_=outt[32 * c : 32 * c + C, :],
                    )
    nc.finalize()
    return nc


def _prepare(feats, edges, W1, b1, u1, W2, b2, u2):
    """Host-side prep: sigma scaling, edge partitioning, index/weight packing."""
    feats = np.ascontiguousarray(feats, dtype=np.float32)
    edges = np.asarray(edges).reshape(-1, 3)
    src = np.clip(edges[:, 0], 0, N_NODES - 1).astype(np.int64)
    tgt = np.clip(edges[:, 2], 0, N_NODES - 1).astype(np.int64)
    typ = edges[:, 1]

    s1 = _sigma(np.asarray(W1, np.float32), np.asarray(u1, np.float32))
    s2 = _sigma(np.asarray(W2, np.float32), np.asarray(u2, np.float32))
    W1s = (np.asarray(W1, np.float32) / s1).astype(np.float32)  # [32,48,3,3]
    W2s = (np.asarray(W2, np.float32) / s2).astype(np.float32)  # [16,32,3,3]

    # per (target node, sign) source lists
    lists = [[[] for _ in range(2)] for _ in range(N_NODES)]
    for e in range(len(src)):
        t = typ[e]
        if t > 0:
            lists[tgt[e]][0].append(int(src[e]))
        elif t < 0:
            lists[tgt[e]][1].append(int(src[e]))
    R = max(1, max(len(l) for node in lists for l in node))

    # gather index tensors per core: [128, NG*R]
    gidx_cores = []
    for core in range(N_CORES):
        gi = np.full((128, NG * R), ZROW, dtype=np.int32)
        for g in range(NG):
            for c in range(4):
                n = core * NPC + g * 4 + c
                if n >= (core + 1) * NPC:
                    continue
                for s in range(2):
                    L = lists[n][s]
                    for k, sv in enumerate(L):
                        gi[32 * c + 16 * s : 32 * c + 16 * s + C, g * R + k] = (
                            sv * C + np.arange(C)
                        )
        gidx_cores.append(gi)

    # weight packing (replicated across the 4 strips)
    o_of = lambda dy, dx: 3 * (dy + 1) + (dx + 1)
    w1p = np.zeros((128, 9, 32), np.float32)
    w1f = np.zeros((128, 9, 32), np.float32)
    w2w = np.zeros((128, 9, 32), np.float32)
    w1bm = np.zeros((128, 1), np.float32)
    w2bm = np.zeros((128, 1), np.float32)
    for c in range(4):
        b = 32 * c
        for dy in (-1, 0, 1):
            for dx in (-1, 0, 1):
                o = o_of(dy, dx)
                # lhsT[j, m] with contraction j, out channel m
                w1p[b : b + 32, o, :] = W1s[:, 16:48, dy + 1, dx + 1].T
                w1f[b : b + C, o, :] = W1s[:, 0:16, dy + 1, dx + 1].T
                w2w[b : b + 32, o, :16] = W2s[:, :, dy + 1, dx + 1].T
        w1bm[b : b + 32, 0] = np.asarray(b1, np.float32)
        w2bm[b : b + C, 0] = np.asarray(b2, np.float32)

    import ml_dtypes
    feats_rows = np.vstack([feats.reshape(ROWS, PIX), np.zeros((1, PIX), np.float32)]).astype(ml_dtypes.bfloat16)
    in_maps = []
    for core in range(N_CORES):
        floc = np.zeros((NPC_PAD, C, HP, HP), np.float32)
        floc[:NPC, :, 1 : 1 + H, 1 : 1 + W] = feats.reshape(N_NODES, C, H, W)[
            core * NPC : (core + 1) * NPC
        ]
        floc = floc.reshape(NPC_PAD, C, HP * HP).astype(ml_dtypes.bfloat16)
        fres = np.zeros((NPC_PAD, C, PIX), np.float32)
        fres[:NPC] = feats.reshape(N_NODES, C, PIX)[core * NPC : (core + 1) * NPC]
        in_maps.append(
            {
                "feats_rows": feats_rows,
                "feats_loc": floc,
                "feats_res": fres,
                "gidx": gidx_cores[core],
                "w1p": w1p.reshape(128, 9 * 32).astype(ml_dtypes.bfloat16),
                "w1f": w1f.reshape(128, 9 * 32).astype(ml_dtypes.bfloat16),
                "w2w": w2w.reshape(128, 9 * 32).astype(ml_dtypes.bfloat16),
                "w1b": w1bm,
                "w2b": w2bm,
            }
        )
    return in_maps, R


def _get_program(R):
    global _compiled
    if _compiled is None or _compiled[1] != R:
        _compiled = (_build(R), R)
    return _compiled[0]


def kernel(**inputs) -> np.ndarray:
    in_maps, R = _prepare(**inputs)
    nc = _get_program(R)
    res = run_bass_kernel_spmd(nc, in_maps, core_ids=list(range(N_CORES)))
    out = np.empty((N_NODES, C, H, W), dtype=np.float32)
    for core in range(N_CORES):
        out[core * NPC : (core + 1) * NPC] = res.results[core]["out"][:NPC].reshape(
            NPC, C, H, W
        )
    return out


if __name__ == "__main__":
    import reference

    ins = {k: np.asarray(v) for k, v in reference.setup_inputs().items()}
    got = kernel(**ins)
    exp = np.asarray(reference.reference(**reference.setup_inputs()))
    rel = np.linalg.norm(got - exp) / np.linalg.norm(exp)
    print("Relative error:", rel)


# revision 22
# speedup vs baseline: 1.7201x; 1.7201x over previous
"""Trainium2 Bass kernel for gnn_message_passing (nn_CMP_32427003085021).

reference semantics:
    pooled_pos[t] = sum_{e: tgt=t, typ>0} feats[src_e]
    pooled_neg[t] = sum_{e: tgt=t, typ<0} feats[src_e]
    combined = [feats; pooled_pos; pooled_neg]            # [N, 48, 32, 32]
    h = lrelu(snconv(combined, W1/s1) + b1, 0.2)
    h = lrelu(snconv(h, W2/s2) + b2, 0.2)
    out = feats + h

Distribution: 8 cores, 250 target-nodes each (data parallel over nodes).
Full feats replicated per core (local HBM) so every gather is core-local.
Pooling is done entirely by SWDGE indirect accumulate-gather DMAs
(compute_op=add, OOB-padded index rounds) writing into padded SBUF images.
Convs are kn2row: 9 shifted matmuls (fp32r, full PE column rate) per output
half, with 4 nodes running concurrently on disjoint 32-row/32-col PE
sub-array tiles (diagonal tile_position via base partitions).
"""
import sys

if "/opt/trn_rl_repo" not in sys.path:
    sys.path.insert(0, "/opt/trn_rl_repo")

import numpy as np

from concourse import bass, bacc, mybir
import concourse.tile as tile
from concourse.bass_utils import run_bass_kernel_spmd

# problem constants (hardcoded per contract)
N_NODES = 2000
N_EDGES = 8000
C = 16
H = W = 32
PIX = H * W          # 1024
HP = H + 2           # 34
N_CORES = 8
NPC = N_NODES // N_CORES        # 250
NPC_PAD = 252                   # multiple of 4
NG = NPC_PAD // 4               # 63 groups of 4 nodes
ROWS = N_NODES * C              # gather row space: feats as [32000, 1024]
ZROW = ROWS                     # appended all-zeros row; pad gathers add 0

F32 = mybir.dt.float32
F32R = mybir.dt.float32r
BF16 = mybir.dt.bfloat16
I32 = mybir.dt.int32

_compiled = None  # (nc, R) cache


def _l2n(x):
    return x / (np.linalg.norm(x) + 1e-12)


def _sigma(Wc, u):
    """One power iteration, mirrors reference fp32 math."""
    O = Wc.shape[0]
    Wm = Wc.reshape(O, -1).astype(np.float32)
    v = _l2n(Wm.T @ u.astype(np.float32))
    u2 = _l2n(Wm @ v)
    return float(u2 @ (Wm @ v))


def _build(RG):
    """Build the SPMD program. RG[g] = gather rounds for group g (degree-sorted)."""
    RTOT = sum(RG)
    col_base = np.cumsum([0] + list(RG))
    nc = bacc.Bacc("TRN2", target_bir_lowering=False, debug=False)
    feats_rows = nc.dram_tensor("feats_rows", [ROWS + 1, PIX], BF16, kind="ExternalInput")
    feats_loc = nc.dram_tensor("feats_loc", [NPC_PAD, C, HP * HP], BF16, kind="ExternalInput")
    feats_res = nc.dram_tensor("feats_res", [NPC_PAD, C, PIX], F32, kind="ExternalInput")
    gidx = nc.dram_tensor("gidx", [128, max(1, RTOT)], I32, kind="ExternalInput")
    w1p = nc.dram_tensor("w1p", [128, 9 * 32], BF16, kind="ExternalInput")
    w1f = nc.dram_tensor("w1f", [128, 9 * 32], BF16, kind="ExternalInput")
    w2w = nc.dram_tensor("w2w", [128, 9 * 32], BF16, kind="ExternalInput")
    w1b = nc.dram_tensor("w1b", [128, 1], F32, kind="ExternalInput")
    w2b = nc.dram_tensor("w2b", [128, 1], F32, kind="ExternalInput")
    out = nc.dram_tensor("out", [NPC_PAD, C, PIX], F32, kind="ExternalOutput")

    BUFS = 4
    with tile.TileContext(nc) as tc:
        with (
            tc.tile_pool(name="const", bufs=1) as cpool,
            tc.tile_pool(name="work", bufs=1) as wpool,
            tc.tile_pool(name="psum", bufs=2, space="PSUM") as pspool,
        ):
            # ---- constants resident in SBUF ----
            idx_t = cpool.tile([128, max(1, RTOT)], I32)
            nc.sync.dma_start(out=idx_t[:], in_=gidx[:])
            w1p_t = cpool.tile([128, 9, 32], BF16)
            nc.sync.dma_start(out=w1p_t[:], in_=w1p[:])
            w1f_t = cpool.tile([128, 9, 32], BF16)
            nc.sync.dma_start(out=w1f_t[:], in_=w1f[:])
            w2_t = cpool.tile([128, 9, 32], BF16)
            nc.sync.dma_start(out=w2_t[:], in_=w2w[:])
            w1b_t = cpool.tile([128, 1], F32)
            nc.sync.dma_start(out=w1b_t[:], in_=w1b[:])
            w2b_t = cpool.tile([128, 1], F32)
            nc.sync.dma_start(out=w2b_t[:], in_=w2b[:])
            alpha_t = cpool.tile([128, 1], F32)
            nc.vector.memset(alpha_t[:], 0.2)

            offsets = [(dy, dx) for dy in (-1, 0, 1) for dx in (-1, 0, 1)]

            for g in range(NG):
                comb = wpool.tile([128, HP, HP], BF16, tag="comb", name="comb", bufs=BUFS)
                comb_raw = wpool.tile([128, PIX], BF16, tag="comb_raw", name="comb_raw", bufs=BUFS)
                ft = wpool.tile([128, HP, HP], BF16, tag="ft", name="ft", bufs=BUFS)
                fres = wpool.tile([128, PIX], F32, tag="fres", name="fres", bufs=BUFS)
                h1 = wpool.tile([128, HP, HP], BF16, tag="h1", name="h1", bufs=BUFS)
                h2t = wpool.tile([128, PIX], F32, tag="h2", name="h2", bufs=3)
                outt = wpool.tile([128, PIX], F32, tag="outt", name="outt", bufs=3)
                # zero padded tiles (comb_raw accumulates from zero)
                nc.vector.memset(comb[:], 0.0)
                nc.vector.memset(comb_raw[:], 0.0)
                nc.vector.memset(h1[:], 0.0)
                nc.vector.memset(fres[:], 0.0)

                # self features for the 4 nodes (source pre-padded on host)
                for c in range(4):
                    nc.scalar.dma_start(
                        out=ft[32 * c : 32 * c + C, :, :],
                        in_=feats_loc[4 * g + c],
                    )
                    nc.scalar.dma_start(
                        out=fres[32 * c : 32 * c + C, :],
                        in_=feats_res[4 * g + c],
                    )

                # pooling: RG[g] rounds of indirect accumulate-gather
                for k in range(RG[g]):
                    col = int(col_base[g]) + k
                    nc.gpsimd.indirect_dma_start(
                        out=comb_raw[:],
                        out_offset=None,
                        in_=feats_rows[:],
                        in_offset=bass.IndirectOffsetOnAxis(
                            ap=idx_t[:, col : col + 1], axis=0
                        ),
                        compute_op=mybir.AluOpType.add,
                    )

                # pooled -> padded interior (indirect DMA needs contiguous dest)
                nc.vector.tensor_copy(
                    out=comb[:, 1 : 1 + H, 1 : 1 + W], in_=comb_raw[:]
                )

                # ---- conv1 into ps1 (K=32 pooled + K=16 feats + bias) ----
                ps1 = pspool.tile([128, PIX], F32, tag="ps1", name="ps1")
                for h in range(2):
                    for i, (dy, dx) in enumerate(offsets):
                        r0 = 16 * h + dy + 1
                        c0 = dx + 1
                        for c in range(4):
                            b = 32 * c
                            nc.tensor.matmul(
                                out=ps1[b : b + 32, 512 * h : 512 * h + 512],
                                lhsT=w1p_t[b : b + 32, i, :],
                                rhs=comb[b : b + 32, r0 : r0 + 16, c0 : c0 + W],
                                start=(i == 0),
                                stop=False,
                                tile_position=(b, b),
                                skip_group_check=True,
                            )
                    for i, (dy, dx) in enumerate(offsets):
                        r0 = 16 * h + dy + 1
                        c0 = dx + 1
                        for c in range(4):
                            b = 32 * c
                            nc.tensor.matmul(
                                out=ps1[b : b + 32, 512 * h : 512 * h + 512],
                                lhsT=w1f_t[b : b + C, i, :],
                                rhs=ft[b : b + C, r0 : r0 + 16, c0 : c0 + W],
                                start=False,
                                stop=(i == 8),
                                tile_position=(b, b),
                                skip_group_check=True,
                            )
                    # evac half h with leaky relu into padded h1 interior
                    nc.scalar.activation(
                        out=h1[:, 1 + 16 * h : 17 + 16 * h, 1 : 1 + W],
                        in_=ps1[:, 512 * h : 512 * h + 512],
                        func=mybir.ActivationFunctionType.Prelu,
                        bias=w1b_t[:],
                        scale=1.0,
                        alpha=alpha_t[:],
                    )

                # ---- conv2 into ps2 (K=32 + bias) ----
                ps2 = pspool.tile([128, PIX], F32, tag="ps2", name="ps2")
                for h in range(2):
                    for i, (dy, dx) in enumerate(offsets):
                        r0 = 16 * h + dy + 1
                        c0 = dx + 1
                        for c in range(4):
                            b = 32 * c
                            nc.tensor.matmul(
                                out=ps2[b : b + 32, 512 * h : 512 * h + 512],
                                lhsT=w2_t[b : b + 32, i, :],
                                rhs=h1[b : b + 32, r0 : r0 + 16, c0 : c0 + W],
                                start=(i == 0),
                                stop=(i == 8),
                                tile_position=(b, b),
                                skip_group_check=True,
                            )
                # leaky on full partitions (junk rows unused later)
                nc.scalar.activation(
                    out=h2t[:],
                    in_=ps2[:],
                    func=mybir.ActivationFunctionType.Prelu,
                    bias=w2b_t[:],
                    scale=1.0,
                    alpha=alpha_t[:],
                )
                # residual: out = feats + h  (ft unused rows are zero)
                nc.vector.tensor_tensor(
                    out=outt[:],
                    in0=h2t[:],
                    in1=fres[:],
                    op=mybir.AluOpType.add,
                )
                for c in range(4):
                    nc.scalar.dma_start(
                        out=out[4 * g + c],
                        in_=outt[32 * c : 32 * c + C, :],
                    )
    nc.finalize()
    return nc


def _prepare(feats, edges, W1, b1, u1, W2, b2, u2):
    """Host-side prep: sigma scaling, edge partitioning, index/weight packing."""
    feats = np.ascontiguousarray(feats, dtype=np.float32)
    edges = np.asarray(edges).reshape(-1, 3)
    src = np.clip(edges[:, 0], 0, N_NODES - 1).astype(np.int64)
    tgt = np.clip(edges[:, 2], 0, N_NODES - 1).astype(np.int64)
    typ = edges[:, 1]

    s1 = _sigma(np.asarray(W1, np.float32), np.asarray(u1, np.float32))
    s2 = _sigma(np.asarray(W2, np.float32), np.asarray(u2, np.float32))
    W1s = (np.asarray(W1, np.float32) / s1).astype(np.float32)  # [32,48,3,3]
    W2s = (np.asarray(W2, np.float32) / s2).astype(np.float32)  # [16,32,3,3]

    # per (target node, sign) source lists
    lists = [[[] for _ in range(2)] for _ in range(N_NODES)]
    for e in range(len(src)):
        t = typ[e]
        if t > 0:
            lists[tgt[e]][0].append(int(src[e]))
        elif t < 0:
            lists[tgt[e]][1].append(int(src[e]))

    # degree-sort nodes per core (descending max-sign-degree); the node
    # permutation is per-core DATA, the per-group round counts RG must be
    # shared across cores (one SPMD program)
    perms = []
    for core in range(N_CORES):
        keys = np.array(
            [max(len(lists[core * NPC + i][0]), len(lists[core * NPC + i][1]))
             for i in range(NPC)]
        )
        perm = np.argsort(-keys, kind="stable")
        perms.append(perm)
    RG = []
    for g in range(NG):
        r = 0
        for core in range(N_CORES):
            for c in range(4):
                i = g * 4 + c
                if i < NPC:
                    n = core * NPC + int(perms[core][i])
                    r = max(r, len(lists[n][0]), len(lists[n][1]))
        RG.append(r)
    RTOT = sum(RG)
    col_base = np.cumsum([0] + RG)

    gidx_cores = []
    for core in range(N_CORES):
        gi = np.full((128, max(1, RTOT)), ZROW, dtype=np.int32)
        for g in range(NG):
            for c in range(4):
                i = g * 4 + c
                if i >= NPC:
                    continue
                n = core * NPC + int(perms[core][i])
                for s in range(2):
                    L = lists[n][s]
                    for k, sv in enumerate(L):
                        gi[32 * c + 16 * s : 32 * c + 16 * s + C,
                           int(col_base[g]) + k] = sv * C + np.arange(C)
        gidx_cores.append(gi)

    # weight packing (replicated across the 4 strips)
    o_of = lambda dy, dx: 3 * (dy + 1) + (dx + 1)
    w1p = np.zeros((128, 9, 32), np.float32)
    w1f = np.zeros((128, 9, 32), np.float32)
    w2w = np.zeros((128, 9, 32), np.float32)
    w1bm = np.zeros((128, 1), np.float32)
    w2bm = np.zeros((128, 1), np.float32)
    for c in range(4):
        b = 32 * c
        for dy in (-1, 0, 1):
            for dx in (-1, 0, 1):
                o = o_of(dy, dx)
                # lhsT[j, m] with contraction j, out channel m
                w1p[b : b + 32, o, :] = W1s[:, 16:48, dy + 1, dx + 1].T
                w1f[b : b + C, o, :] = W1s[:, 0:16, dy + 1, dx + 1].T
                w2w[b : b + 32, o, :16] = W2s[:, :, dy + 1, dx + 1].T
        w1bm[b : b + 32, 0] = np.asarray(b1, np.float32)
        w2bm[b : b + C, 0] = np.asarray(b2, np.float32)

    import ml_dtypes
    feats_rows = np.vstack([feats.reshape(ROWS, PIX), np.zeros((1, PIX), np.float32)]).astype(ml_dtypes.bfloat16)
    in_maps = []
    for core in range(N_CORES):
        fcore = feats.reshape(N_NODES, C, H, W)[core * NPC : (core + 1) * NPC]
        fcore = fcore[perms[core]]
        floc = np.zeros((NPC_PAD, C, HP, HP), np.float32)
        floc[:NPC, :, 1 : 1 + H, 1 : 1 + W] = fcore
        floc = floc.reshape(NPC_PAD, C, HP * HP).astype(ml_dtypes.bfloat16)
        fres = np.zeros((NPC_PAD, C, PIX), np.float32)
        fres[:NPC] = fcore.reshape(NPC, C, PIX)
        in_maps.append(
            {
                "feats_rows": feats_rows,
                "feats_loc": floc,
                "feats_res": fres,
                "gidx": gidx_cores[core],
                "w1p": w1p.reshape(128, 9 * 32).astype(ml_dtypes.bfloat16),
                "w1f": w1f.reshape(128, 9 * 32).astype(ml_dtypes.bfloat16),
                "w2w": w2w.reshape(128, 9 * 32).astype(ml_dtypes.bfloat16),
                "w1b": w1bm,
                "w2b": w2bm,
            }
        )
    return in_maps, (tuple(RG), perms)


def _get_program(key):
    global _compiled
    RG = key[0]
    if _compiled is None or _compiled[1] != RG:
        _compiled = (_build(list(RG)), RG)
    return _compiled[0]


def kernel(**inputs) -> np.ndarray:
    in_maps, key = _prepare(**inputs)
    nc = _get_program(key)
    perms = key[1]
    res = run_bass_kernel_spmd(nc, in_maps, core_ids=list(range(N_CORES)))
    out = np.empty((N_NODES, C, H, W), dtype=np.float32)
    for core in range(N_CORES):
        blk = res.results[core]["out"][:NPC].reshape(NPC, C, H, W)
        out[core * NPC + perms[core]] = blk
    return out


if __name__ == "__main__":
    import reference

    ins = {k: np.asarray(v) for k, v in reference.setup_inputs().items()}
    got = kernel(**ins)
    exp = np.asarray(reference.reference(**reference.setup_inputs()))
    rel = np.linalg.norm(got - exp) / np.linalg.norm(exp)
    print("Relative error:", rel)


# revision 23
# speedup vs baseline: 2.5039x; 1.4557x over previous
"""Trainium2 Bass kernel for gnn_message_passing (nn_CMP_32427003085021).

reference semantics:
    pooled_pos[t] = sum_{e: tgt=t, typ>0} feats[src_e]
    pooled_neg[t] = sum_{e: tgt=t, typ<0} feats[src_e]
    combined = [feats; pooled_pos; pooled_neg]            # [N, 48, 32, 32]
    h = lrelu(snconv(combined, W1/s1) + b1, 0.2)
    h = lrelu(snconv(h, W2/s2) + b2, 0.2)
    out = feats + h

Distribution: 8 cores, 250 target-nodes each (data parallel over nodes).
Full feats replicated per core (local HBM) so every gather is core-local.
Pooling is done entirely by SWDGE indirect accumulate-gather DMAs
(compute_op=add, OOB-padded index rounds) writing into padded SBUF images.
Convs are kn2row: 9 shifted matmuls (fp32r, full PE column rate) per output
half, with 4 nodes running concurrently on disjoint 32-row/32-col PE
sub-array tiles (diagonal tile_position via base partitions).
"""
import sys

if "/opt/trn_rl_repo" not in sys.path:
    sys.path.insert(0, "/opt/trn_rl_repo")

import numpy as np

from concourse import bass, bacc, mybir
import concourse.tile as tile
from concourse.bass_utils import run_bass_kernel_spmd

# problem constants (hardcoded per contract)
N_NODES = 2000
N_EDGES = 8000
C = 16
H = W = 32
PIX = H * W          # 1024
HP = H + 2           # 34
N_CORES = 8
NPC = N_NODES // N_CORES        # 250
NPC_PAD = 252                   # multiple of 4
NG = NPC_PAD // 4               # 63 groups of 4 nodes
ROWS = N_NODES * C              # gather row space: feats as [32000, 1024]
ZROW = ROWS                     # appended all-zeros row; pad gathers add 0

F32 = mybir.dt.float32
F32R = mybir.dt.float32r
BF16 = mybir.dt.bfloat16
I32 = mybir.dt.int32

_compiled = None  # (nc, R) cache


def _l2n(x):
    return x / (np.linalg.norm(x) + 1e-12)


def _sigma(Wc, u):
    """One power iteration, mirrors reference fp32 math."""
    O = Wc.shape[0]
    Wm = Wc.reshape(O, -1).astype(np.float32)
    v = _l2n(Wm.T @ u.astype(np.float32))
    u2 = _l2n(Wm @ v)
    return float(u2 @ (Wm @ v))


def _build(RG):
    """Build the SPMD program. RG[g] = gather rounds for group g (degree-sorted)."""
    RTOT = sum(RG)
    col_base = np.cumsum([0] + list(RG))
    nc = bacc.Bacc("TRN2", target_bir_lowering=False, debug=False)
    feats_rows = nc.dram_tensor("feats_rows", [ROWS + 1, PIX], BF16, kind="ExternalInput")
    feats_loc = nc.dram_tensor("feats_loc", [NPC_PAD, C, HP * HP], BF16, kind="ExternalInput")
    feats_res = nc.dram_tensor("feats_res", [NPC_PAD, C, PIX], F32, kind="ExternalInput")
    gidx = nc.dram_tensor("gidx", [128, max(1, RTOT)], I32, kind="ExternalInput")
    w1p = nc.dram_tensor("w1p", [128, 9 * 32], BF16, kind="ExternalInput")
    w1f = nc.dram_tensor("w1f", [128, 9 * 32], BF16, kind="ExternalInput")
    w2w = nc.dram_tensor("w2w", [128, 9 * 32], BF16, kind="ExternalInput")
    w1b = nc.dram_tensor("w1b", [128, 1], F32, kind="ExternalInput")
    w2b = nc.dram_tensor("w2b", [128, 1], F32, kind="ExternalInput")
    out = nc.dram_tensor("out", [NPC_PAD, C, PIX], F32, kind="ExternalOutput")

    BUFS = 6
    with tile.TileContext(nc) as tc:
        with (
            tc.tile_pool(name="const", bufs=1) as cpool,
            tc.tile_pool(name="work", bufs=1) as wpool,
            tc.tile_pool(name="psum", bufs=2, space="PSUM") as pspool,
        ):
            # ---- constants resident in SBUF ----
            idx_t = cpool.tile([128, max(1, RTOT)], I32)
            nc.sync.dma_start(out=idx_t[:], in_=gidx[:])
            w1p_t = cpool.tile([128, 9, 32], BF16)
            nc.sync.dma_start(out=w1p_t[:], in_=w1p[:])
            w1f_t = cpool.tile([128, 9, 32], BF16)
            nc.sync.dma_start(out=w1f_t[:], in_=w1f[:])
            w2_t = cpool.tile([128, 9, 32], BF16)
            nc.sync.dma_start(out=w2_t[:], in_=w2w[:])
            w1b_t = cpool.tile([128, 1], F32)
            nc.sync.dma_start(out=w1b_t[:], in_=w1b[:])
            w2b_t = cpool.tile([128, 1], F32)
            nc.sync.dma_start(out=w2b_t[:], in_=w2b[:])
            alpha_t = cpool.tile([128, 1], F32)
            nc.vector.memset(alpha_t[:], 0.2)

            offsets = [(dy, dx) for dy in (-1, 0, 1) for dx in (-1, 0, 1)]

            for g in range(NG):
                comb = wpool.tile([128, HP, HP], BF16, tag="comb", name="comb", bufs=BUFS)
                comb_raw = wpool.tile([128, PIX], BF16, tag="comb_raw", name="comb_raw", bufs=BUFS)
                ft = wpool.tile([128, HP, HP], BF16, tag="ft", name="ft", bufs=BUFS)
                fres = wpool.tile([128, PIX], F32, tag="fres", name="fres", bufs=BUFS)
                h1 = wpool.tile([128, HP, HP], BF16, tag="h1", name="h1", bufs=BUFS)
                h2t = wpool.tile([128, PIX], F32, tag="h2", name="h2", bufs=3)
                outt = wpool.tile([128, PIX], F32, tag="outt", name="outt", bufs=3)
                # zero padded tiles (comb_raw accumulates from zero)
                nc.vector.memset(comb[:], 0.0)
                nc.vector.memset(comb_raw[:], 0.0)
                nc.vector.memset(h1[:], 0.0)
                nc.vector.memset(fres[:], 0.0)

                # self features for the 4 nodes (source pre-padded on host)
                for c in range(4):
                    nc.sync.dma_start(
                        out=ft[32 * c : 32 * c + C, :, :],
                        in_=feats_loc[4 * g + c],
                    )
                    nc.sync.dma_start(
                        out=fres[32 * c : 32 * c + C, :],
                        in_=feats_res[4 * g + c],
                    )

                # pooling: RG[g] rounds of indirect accumulate-gather
                for k in range(RG[g]):
                    col = int(col_base[g]) + k
                    nc.gpsimd.indirect_dma_start(
                        out=comb_raw[:],
                        out_offset=None,
                        in_=feats_rows[:],
                        in_offset=bass.IndirectOffsetOnAxis(
                            ap=idx_t[:, col : col + 1], axis=0
                        ),
                        compute_op=mybir.AluOpType.add,
                    )

                # pooled -> padded interior (indirect DMA needs contiguous dest)
                nc.vector.tensor_copy(
                    out=comb[:, 1 : 1 + H, 1 : 1 + W], in_=comb_raw[:]
                )

                # ---- conv1 into ps1 (K=32 pooled + K=16 feats + bias) ----
                ps1 = pspool.tile([128, PIX], F32, tag="ps1", name="ps1")
                for h in range(2):
                    for i, (dy, dx) in enumerate(offsets):
                        r0 = 16 * h + dy + 1
                        c0 = dx + 1
                        for c in range(4):
                            b = 32 * c
                            nc.tensor.matmul(
                                out=ps1[b : b + 32, 512 * h : 512 * h + 512],
                                lhsT=w1p_t[b : b + 32, i, :],
                                rhs=comb[b : b + 32, r0 : r0 + 16, c0 : c0 + W],
                                start=(i == 0),
                                stop=False,
                                tile_position=(b, b),
                                skip_group_check=True,
                            )
                    for i, (dy, dx) in enumerate(offsets):
                        r0 = 16 * h + dy + 1
                        c0 = dx + 1
                        for c in range(4):
                            b = 32 * c
                            nc.tensor.matmul(
                                out=ps1[b : b + 32, 512 * h : 512 * h + 512],
                                lhsT=w1f_t[b : b + C, i, :],
                                rhs=ft[b : b + C, r0 : r0 + 16, c0 : c0 + W],
                                start=False,
                                stop=(i == 8),
                                tile_position=(b, b),
                                skip_group_check=True,
                            )
                    # evac half h with leaky relu into padded h1 interior
                    nc.scalar.activation(
                        out=h1[:, 1 + 16 * h : 17 + 16 * h, 1 : 1 + W],
                        in_=ps1[:, 512 * h : 512 * h + 512],
                        func=mybir.ActivationFunctionType.Prelu,
                        bias=w1b_t[:],
                        scale=1.0,
                        alpha=alpha_t[:],
                    )

                # ---- conv2 into ps2 (K=32 + bias) ----
                ps2 = pspool.tile([128, PIX], F32, tag="ps2", name="ps2")
                for h in range(2):
                    for i, (dy, dx) in enumerate(offsets):
                        r0 = 16 * h + dy + 1
                        c0 = dx + 1
                        for c in range(4):
                            b = 32 * c
                            nc.tensor.matmul(
                                out=ps2[b : b + 32, 512 * h : 512 * h + 512],
                                lhsT=w2_t[b : b + 32, i, :],
                                rhs=h1[b : b + 32, r0 : r0 + 16, c0 : c0 + W],
                                start=(i == 0),
                                stop=(i == 8),
                                tile_position=(b, b),
                                skip_group_check=True,
                            )
                # leaky on full partitions (junk rows unused later)
                nc.scalar.activation(
                    out=h2t[:],
                    in_=ps2[:],
                    func=mybir.ActivationFunctionType.Prelu,
                    bias=w2b_t[:],
                    scale=1.0,
                    alpha=alpha_t[:],
                )
                # residual: out = feats + h  (ft unused rows are zero)
                nc.vector.tensor_tensor(
                    out=outt[:],
                    in0=h2t[:],
                    in1=fres[:],
                    op=mybir.AluOpType.add,
                )
                for c in range(4):
                    nc.sync.dma_start(
                        out=out[4 * g + c],
                        in_=outt[32 * c : 32 * c + C, :],
                    )
    nc.finalize()
    return nc


def _prepare(feats, edges, W1, b1, u1, W2, b2, u2):
    """Host-side prep: sigma scaling, edge partitioning, index/weight packing."""
    feats = np.ascontiguousarray(feats, dtype=np.float32)
    edges = np.asarray(edges).reshape(-1, 3)
    src = np.clip(edges[:, 0], 0, N_NODES - 1).astype(np.int64)
    tgt = np.clip(edges[:, 2], 0, N_NODES - 1).astype(np.int64)
    typ = edges[:, 1]

    s1 = _sigma(np.asarray(W1, np.float32), np.asarray(u1, np.float32))
    s2 = _sigma(np.asarray(W2, np.float32), np.asarray(u2, np.float32))
    W1s = (np.asarray(W1, np.float32) / s1).astype(np.float32)  # [32,48,3,3]
    W2s = (np.asarray(W2, np.float32) / s2).astype(np.float32)  # [16,32,3,3]

    # per (target node, sign) source lists
    lists = [[[] for _ in range(2)] for _ in range(N_NODES)]
    for e in range(len(src)):
        t = typ[e]
        if t > 0:
            lists[tgt[e]][0].append(int(src[e]))
        elif t < 0:
            lists[tgt[e]][1].append(int(src[e]))

    # degree-sort nodes per core (descending max-sign-degree); the node
    # permutation is per-core DATA, the per-group round counts RG must be
    # shared across cores (one SPMD program)
    perms = []
    for core in range(N_CORES):
        keys = np.array(
            [max(len(lists[core * NPC + i][0]), len(lists[core * NPC + i][1]))
             for i in range(NPC)]
        )
        perm = np.argsort(-keys, kind="stable")
        perms.append(perm)
    RG = []
    for g in range(NG):
        r = 0
        for core in range(N_CORES):
            for c in range(4):
                i = g * 4 + c
                if i < NPC:
                    n = core * NPC + int(perms[core][i])
                    r = max(r, len(lists[n][0]), len(lists[n][1]))
        RG.append(r)
    RTOT = sum(RG)
    col_base = np.cumsum([0] + RG)

    gidx_cores = []
    for core in range(N_CORES):
        gi = np.full((128, max(1, RTOT)), ZROW, dtype=np.int32)
        for g in range(NG):
            for c in range(4):
                i = g * 4 + c
                if i >= NPC:
                    continue
                n = core * NPC + int(perms[core][i])
                for s in range(2):
                    L = lists[n][s]
                    for k, sv in enumerate(L):
                        gi[32 * c + 16 * s : 32 * c + 16 * s + C,
                           int(col_base[g]) + k] = sv * C + np.arange(C)
        gidx_cores.append(gi)

    # weight packing (replicated across the 4 strips)
    o_of = lambda dy, dx: 3 * (dy + 1) + (dx + 1)
    w1p = np.zeros((128, 9, 32), np.float32)
    w1f = np.zeros((128, 9, 32), np.float32)
    w2w = np.zeros((128, 9, 32), np.float32)
    w1bm = np.zeros((128, 1), np.float32)
    w2bm = np.zeros((128, 1), np.float32)
    for c in range(4):
        b = 32 * c
        for dy in (-1, 0, 1):
            for dx in (-1, 0, 1):
                o = o_of(dy, dx)
                # lhsT[j, m] with contraction j, out channel m
                w1p[b : b + 32, o, :] = W1s[:, 16:48, dy + 1, dx + 1].T
                w1f[b : b + C, o, :] = W1s[:, 0:16, dy + 1, dx + 1].T
                w2w[b : b + 32, o, :16] = W2s[:, :, dy + 1, dx + 1].T
        w1bm[b : b + 32, 0] = np.asarray(b1, np.float32)
        w2bm[b : b + C, 0] = np.asarray(b2, np.float32)

    import ml_dtypes
    feats_rows = np.vstack([feats.reshape(ROWS, PIX), np.zeros((1, PIX), np.float32)]).astype(ml_dtypes.bfloat16)
    in_maps = []
    for core in range(N_CORES):
        fcore = feats.reshape(N_NODES, C, H, W)[core * NPC : (core + 1) * NPC]
        fcore = fcore[perms[core]]
        floc = np.zeros((NPC_PAD, C, HP, HP), np.float32)
        floc[:NPC, :, 1 : 1 + H, 1 : 1 + W] = fcore
        floc = floc.reshape(NPC_PAD, C, HP * HP).astype(ml_dtypes.bfloat16)
        fres = np.zeros((NPC_PAD, C, PIX), np.float32)
        fres[:NPC] = fcore.reshape(NPC, C, PIX)
        in_maps.append(
            {
                "feats_rows": feats_rows,
                "feats_loc": floc,
                "feats_res": fres,
                "gidx": gidx_cores[core],
                "w1p": w1p.reshape(128, 9 * 32).astype(ml_dtypes.bfloat16),
                "w1f": w1f.reshape(128, 9 * 32).astype(ml_dtypes.bfloat16),
                "w2w": w2w.reshape(128, 9 * 32).astype(ml_dtypes.bfloat16),
                "w1b": w1bm,
                "w2b": w2bm,
            }
        )
    return in_maps, (tuple(RG), perms)


def _get_program(key):
    global _compiled
    RG = key[0]
    if _compiled is None or _compiled[1] != RG:
        _compiled = (_build(list(RG)), RG)
    return _compiled[0]


def kernel(**inputs) -> np.ndarray:
    in_maps, key = _prepare(**inputs)
    nc = _get_program(key)
    perms = key[1]
    res = run_bass_kernel_spmd(nc, in_maps, core_ids=list(range(N_CORES)))
    out = np.empty((N_NODES, C, H, W), dtype=np.float32)
    for core in range(N_CORES):
        blk = res.results[core]["out"][:NPC].reshape(NPC, C, H, W)
        out[core * NPC + perms[core]] = blk
    return out


if __name__ == "__main__":
    import reference

    ins = {k: np.asarray(v) for k, v in reference.setup_inputs().items()}
    got = kernel(**ins)
    exp = np.asarray(reference.reference(**reference.setup_inputs()))
    rel = np.linalg.norm(got - exp) / np.linalg.norm(exp)
    print("Relative error:", rel)


# revision 25
# speedup vs baseline: 2.7271x; 1.0891x over previous
"""Trainium2 Bass kernel for gnn_message_passing (nn_CMP_32427003085021).

reference semantics:
    pooled_pos[t] = sum_{e: tgt=t, typ>0} feats[src_e]
    pooled_neg[t] = sum_{e: tgt=t, typ<0} feats[src_e]
    combined = [feats; pooled_pos; pooled_neg]            # [N, 48, 32, 32]
    h = lrelu(snconv(combined, W1/s1) + b1, 0.2)
    h = lrelu(snconv(h, W2/s2) + b2, 0.2)
    out = feats + h

Distribution: 8 cores, 250 target-nodes each (data parallel over nodes).
Full feats replicated per core (local HBM) so every gather is core-local.
Pooling is done entirely by SWDGE indirect accumulate-gather DMAs
(compute_op=add, OOB-padded index rounds) writing into padded SBUF images.
Convs are kn2row: 9 shifted matmuls (fp32r, full PE column rate) per output
half, with 4 nodes running concurrently on disjoint 32-row/32-col PE
sub-array tiles (diagonal tile_position via base partitions).
"""
import sys

if "/opt/trn_rl_repo" not in sys.path:
    sys.path.insert(0, "/opt/trn_rl_repo")

import numpy as np

from concourse import bass, bacc, mybir
import concourse.tile as tile
from concourse.bass_utils import run_bass_kernel_spmd

# problem constants (hardcoded per contract)
N_NODES = 2000
N_EDGES = 8000
C = 16
H = W = 32
PIX = H * W          # 1024
HP = H + 2           # 34
N_CORES = 8
NPC = N_NODES // N_CORES        # 250
NPC_PAD = 252                   # multiple of 4
NG = NPC_PAD // 4               # 63 groups of 4 nodes
ROWS = N_NODES * C              # gather row space: feats as [32000, 1024]
ZROW = ROWS                     # appended all-zeros row; pad gathers add 0

F32 = mybir.dt.float32
F32R = mybir.dt.float32r
BF16 = mybir.dt.bfloat16
I32 = mybir.dt.int32

_compiled = None  # (nc, R) cache


def _l2n(x):
    return x / (np.linalg.norm(x) + 1e-12)


def _sigma(Wc, u):
    """One power iteration, mirrors reference fp32 math."""
    O = Wc.shape[0]
    Wm = Wc.reshape(O, -1).astype(np.float32)
    v = _l2n(Wm.T @ u.astype(np.float32))
    u2 = _l2n(Wm @ v)
    return float(u2 @ (Wm @ v))


def _build(RG):
    """Build the SPMD program. RG[g] = gather rounds for group g (degree-sorted)."""
    RTOT = sum(RG)
    col_base = np.cumsum([0] + list(RG))
    nc = bacc.Bacc("TRN2", target_bir_lowering=False, debug=False)
    feats_rows = nc.dram_tensor("feats_rows", [ROWS + 1, PIX], BF16, kind="ExternalInput")
    feats_loc = nc.dram_tensor("feats_loc", [NPC_PAD, 2 * C, HP * HP], BF16, kind="ExternalInput")
    feats_res = nc.dram_tensor("feats_res", [NPC_PAD, C, PIX], F32, kind="ExternalInput")
    gidx = nc.dram_tensor("gidx", [128, max(1, RTOT)], I32, kind="ExternalInput")
    w1p = nc.dram_tensor("w1p", [128, 9 * 32], BF16, kind="ExternalInput")
    w1f = nc.dram_tensor("w1f", [128, 6 * 32], BF16, kind="ExternalInput")
    w2w = nc.dram_tensor("w2w", [128, 9 * 32], BF16, kind="ExternalInput")
    w1b = nc.dram_tensor("w1b", [128, 1], F32, kind="ExternalInput")
    w2b = nc.dram_tensor("w2b", [128, 1], F32, kind="ExternalInput")
    out = nc.dram_tensor("out", [NPC_PAD, C, PIX], F32, kind="ExternalOutput")

    BUFS = 6
    with tile.TileContext(nc) as tc:
        with (
            tc.tile_pool(name="const", bufs=1) as cpool,
            tc.tile_pool(name="work", bufs=1) as wpool,
            tc.tile_pool(name="psum", bufs=2, space="PSUM") as pspool,
        ):
            # ---- constants resident in SBUF ----
            idx_t = cpool.tile([128, max(1, RTOT)], I32)
            nc.sync.dma_start(out=idx_t[:], in_=gidx[:])
            w1p_t = cpool.tile([128, 9, 32], BF16)
            nc.sync.dma_start(out=w1p_t[:], in_=w1p[:])
            w1f_t = cpool.tile([128, 6, 32], BF16)
            nc.sync.dma_start(out=w1f_t[:], in_=w1f[:])
            w2_t = cpool.tile([128, 9, 32], BF16)
            nc.sync.dma_start(out=w2_t[:], in_=w2w[:])
            w1b_t = cpool.tile([128, 1], F32)
            nc.sync.dma_start(out=w1b_t[:], in_=w1b[:])
            w2b_t = cpool.tile([128, 1], F32)
            nc.sync.dma_start(out=w2b_t[:], in_=w2b[:])
            alpha_t = cpool.tile([128, 1], F32)
            nc.vector.memset(alpha_t[:], 0.2)

            offsets = [(dy, dx) for dy in (-1, 0, 1) for dx in (-1, 0, 1)]

            for g in range(NG):
                comb = wpool.tile([128, HP, HP], BF16, tag="comb", name="comb", bufs=BUFS)
                comb_raw = wpool.tile([128, PIX], BF16, tag="comb_raw", name="comb_raw", bufs=BUFS)
                ft = wpool.tile([128, HP, HP], BF16, tag="ft", name="ft", bufs=BUFS)
                fres = wpool.tile([128, PIX], F32, tag="fres", name="fres", bufs=BUFS)
                h1 = wpool.tile([128, HP, HP], BF16, tag="h1", name="h1", bufs=BUFS)
                h2t = wpool.tile([128, PIX], F32, tag="h2", name="h2", bufs=3)
                outt = wpool.tile([128, PIX], F32, tag="outt", name="outt", bufs=3)
                # zero padded tiles (comb_raw accumulates from zero)
                nc.vector.memset(comb[:], 0.0)
                nc.vector.memset(comb_raw[:], 0.0)
                nc.vector.memset(h1[:], 0.0)
                nc.vector.memset(fres[:], 0.0)

                # self features for the 4 nodes (source pre-padded on host)
                for c in range(4):
                    nc.sync.dma_start(
                        out=ft[32 * c : 32 * c + 2 * C, :, :],
                        in_=feats_loc[4 * g + c],
                    )
                    nc.sync.dma_start(
                        out=fres[32 * c : 32 * c + C, :],
                        in_=feats_res[4 * g + c],
                    )

                # pooling: RG[g] rounds of indirect accumulate-gather
                for k in range(RG[g]):
                    col = int(col_base[g]) + k
                    nc.gpsimd.indirect_dma_start(
                        out=comb_raw[:],
                        out_offset=None,
                        in_=feats_rows[:],
                        in_offset=bass.IndirectOffsetOnAxis(
                            ap=idx_t[:, col : col + 1], axis=0
                        ),
                        compute_op=mybir.AluOpType.add,
                    )

                # pooled -> padded interior (indirect DMA needs contiguous dest)
                nc.vector.tensor_copy(
                    out=comb[:, 1 : 1 + H, 1 : 1 + W], in_=comb_raw[:]
                )

                # ---- conv1 into ps1 (K=32 pooled + K=16 feats + bias) ----
                ps1 = pspool.tile([128, PIX], F32, tag="ps1", name="ps1")
                for h in range(2):
                    for i, (dy, dx) in enumerate(offsets):
                        r0 = 16 * h + dy + 1
                        c0 = dx + 1
                        for c in range(4):
                            b = 32 * c
                            nc.tensor.matmul(
                                out=ps1[b : b + 32, 512 * h : 512 * h + 512],
                                lhsT=w1p_t[b : b + 32, i, :],
                                rhs=comb[b : b + 32, r0 : r0 + 16, c0 : c0 + W],
                                start=(i == 0),
                                stop=False,
                                tile_position=(b, b),
                                skip_group_check=True,
                            )
                    for j in range(6):
                        if j < 3:           # paired: dy in {0,-1} via dual copy, K=32
                            dx = j - 1
                            r0 = 16 * h + 1
                            kk = 32
                        else:               # dy=+1, K=16 on the normal copy
                            dx = j - 4
                            r0 = 16 * h + 2
                            kk = C
                        c0 = dx + 1
                        for c in range(4):
                            b = 32 * c
                            nc.tensor.matmul(
                                out=ps1[b : b + 32, 512 * h : 512 * h + 512],
                                lhsT=w1f_t[b : b + kk, j, :],
                                rhs=ft[b : b + kk, r0 : r0 + 16, c0 : c0 + W],
                                start=False,
                                stop=(j == 5),
                                tile_position=(b, b),
                                skip_group_check=True,
                            )
                    # evac half h with leaky relu into padded h1 interior
                    nc.scalar.activation(
                        out=h1[:, 1 + 16 * h : 17 + 16 * h, 1 : 1 + W],
                        in_=ps1[:, 512 * h : 512 * h + 512],
                        func=mybir.ActivationFunctionType.Prelu,
                        bias=w1b_t[:],
                        scale=1.0,
                        alpha=alpha_t[:],
                    )

                # ---- conv2 into ps2 (K=32 + bias) ----
                ps2 = pspool.tile([128, PIX], F32, tag="ps2", name="ps2")
                for h in range(2):
                    for i, (dy, dx) in enumerate(offsets):
                        r0 = 16 * h + dy + 1
                        c0 = dx + 1
                        for c in range(4):
                            b = 32 * c
                            nc.tensor.matmul(
                                out=ps2[b : b + 32, 512 * h : 512 * h + 512],
                                lhsT=w2_t[b : b + 32, i, :],
                                rhs=h1[b : b + 32, r0 : r0 + 16, c0 : c0 + W],
                                start=(i == 0),
                                stop=(i == 8),
                                tile_position=(b, b),
                                skip_group_check=True,
                            )
                # leaky on full partitions (junk rows unused later)
                nc.scalar.activation(
                    out=h2t[:],
                    in_=ps2[:],
                    func=mybir.ActivationFunctionType.Prelu,
                    bias=w2b_t[:],
                    scale=1.0,
                    alpha=alpha_t[:],
                )
                # residual: out = feats + h  (ft unused rows are zero)
                nc.vector.tensor_tensor(
                    out=outt[:],
                    in0=h2t[:],
                    in1=fres[:],
                    op=mybir.AluOpType.add,
                )
                for c in range(4):
                    nc.sync.dma_start(
                        out=out[4 * g + c],
                        in_=outt[32 * c : 32 * c + C, :],
                    )
    nc.finalize()
    return nc


def _prepare(feats, edges, W1, b1, u1, W2, b2, u2):
    """Host-side prep: sigma scaling, edge partitioning, index/weight packing."""
    feats = np.ascontiguousarray(feats, dtype=np.float32)
    edges = np.asarray(edges).reshape(-1, 3)
    src = np.clip(edges[:, 0], 0, N_NODES - 1).astype(np.int64)
    tgt = np.clip(edges[:, 2], 0, N_NODES - 1).astype(np.int64)
    typ = edges[:, 1]

    s1 = _sigma(np.asarray(W1, np.float32), np.asarray(u1, np.float32))
    s2 = _sigma(np.asarray(W2, np.float32), np.asarray(u2, np.float32))
    W1s = (np.asarray(W1, np.float32) / s1).astype(np.float32)  # [32,48,3,3]
    W2s = (np.asarray(W2, np.float32) / s2).astype(np.float32)  # [16,32,3,3]

    # per (target node, sign) source lists
    lists = [[[] for _ in range(2)] for _ in range(N_NODES)]
    for e in range(len(src)):
        t = typ[e]
        if t > 0:
            lists[tgt[e]][0].append(int(src[e]))
        elif t < 0:
            lists[tgt[e]][1].append(int(src[e]))

    # degree-sort nodes per core (descending max-sign-degree); the node
    # permutation is per-core DATA, the per-group round counts RG must be
    # shared across cores (one SPMD program)
    perms = []
    for core in range(N_CORES):
        keys = np.array(
            [max(len(lists[core * NPC + i][0]), len(lists[core * NPC + i][1]))
             for i in range(NPC)]
        )
        perm = np.argsort(-keys, kind="stable")
        perms.append(perm)
    RG = []
    for g in range(NG):
        r = 0
        for core in range(N_CORES):
            for c in range(4):
                i = g * 4 + c
                if i < NPC:
                    n = core * NPC + int(perms[core][i])
                    r = max(r, len(lists[n][0]), len(lists[n][1]))
        RG.append(r)
    RTOT = sum(RG)
    col_base = np.cumsum([0] + RG)

    gidx_cores = []
    for core in range(N_CORES):
        gi = np.full((128, max(1, RTOT)), ZROW, dtype=np.int32)
        for g in range(NG):
            for c in range(4):
                i = g * 4 + c
                if i >= NPC:
                    continue
                n = core * NPC + int(perms[core][i])
                for s in range(2):
                    L = lists[n][s]
                    for k, sv in enumerate(L):
                        gi[32 * c + 16 * s : 32 * c + 16 * s + C,
                           int(col_base[g]) + k] = sv * C + np.arange(C)
        gidx_cores.append(gi)

    # weight packing (replicated across the 4 strips)
    o_of = lambda dy, dx: 3 * (dy + 1) + (dx + 1)
    w1p = np.zeros((128, 9, 32), np.float32)
    w1f = np.zeros((128, 6, 32), np.float32)
    w2w = np.zeros((128, 9, 32), np.float32)
    w1bm = np.zeros((128, 1), np.float32)
    w2bm = np.zeros((128, 1), np.float32)
    for c in range(4):
        b = 32 * c
        for dy in (-1, 0, 1):
            for dx in (-1, 0, 1):
                o = o_of(dy, dx)
                # lhsT[j, m] with contraction j, out channel m
                w1p[b : b + 32, o, :] = W1s[:, 16:48, dy + 1, dx + 1].T
                w2w[b : b + 32, o, :16] = W2s[:, :, dy + 1, dx + 1].T
        for dx in (-1, 0, 1):
            # paired (dy=0 on rows 0-15, dy=-1 on rows 16-31), then dy=+1
            w1f[b : b + C, dx + 1, :] = W1s[:, 0:16, 1, dx + 1].T
            w1f[b + C : b + 32, dx + 1, :] = W1s[:, 0:16, 0, dx + 1].T
            w1f[b : b + C, dx + 4, :] = W1s[:, 0:16, 2, dx + 1].T
        w1bm[b : b + 32, 0] = np.asarray(b1, np.float32)
        w2bm[b : b + C, 0] = np.asarray(b2, np.float32)

    import ml_dtypes
    feats_rows = np.vstack([feats.reshape(ROWS, PIX), np.zeros((1, PIX), np.float32)]).astype(ml_dtypes.bfloat16)
    in_maps = []
    for core in range(N_CORES):
        fcore = feats.reshape(N_NODES, C, H, W)[core * NPC : (core + 1) * NPC]
        fcore = fcore[perms[core]]
        floc = np.zeros((NPC_PAD, 2 * C, HP, HP), np.float32)
        floc[:NPC, :C, 1 : 1 + H, 1 : 1 + W] = fcore          # normal pad
        floc[:NPC, C:, 2 : 2 + H, 1 : 1 + W] = fcore          # shifted down 1 row
        floc = floc.reshape(NPC_PAD, 2 * C, HP * HP).astype(ml_dtypes.bfloat16)
        fres = np.zeros((NPC_PAD, C, PIX), np.float32)
        fres[:NPC] = fcore.reshape(NPC, C, PIX)
        in_maps.append(
            {
                "feats_rows": feats_rows,
                "feats_loc": floc,
                "feats_res": fres,
                "gidx": gidx_cores[core],
                "w1p": w1p.reshape(128, 9 * 32).astype(ml_dtypes.bfloat16),
                "w1f": w1f.reshape(128, 6 * 32).astype(ml_dtypes.bfloat16),
                "w2w": w2w.reshape(128, 9 * 32).astype(ml_dtypes.bfloat16),
                "w1b": w1bm,
                "w2b": w2bm,
            }
        )
    return in_maps, (tuple(RG), perms)


def _get_program(key):
    global _compiled
    RG = key[0]
    if _compiled is None or _compiled[1] != RG:
        _compiled = (_build(list(RG)), RG)
    return _compiled[0]


def kernel(**inputs) -> np.ndarray:
    in_maps, key = _prepare(**inputs)
    nc = _get_program(key)
    perms = key[1]
    res = run_bass_kernel_spmd(nc, in_maps, core_ids=list(range(N_CORES)))
    out = np.empty((N_NODES, C, H, W), dtype=np.float32)
    for core in range(N_CORES):
        blk = res.results[core]["out"][:NPC].reshape(NPC, C, H, W)
        out[core * NPC + perms[core]] = blk
    return out


if __name__ == "__main__":
    import reference

    ins = {k: np.asarray(v) for k, v in reference.setup_inputs().items()}
    got = kernel(**ins)
    exp = np.asarray(reference.reference(**reference.setup_inputs()))
    rel = np.linalg.norm(got - exp) / np.linalg.norm(exp)
    print("Relative error:", rel)
